# revision 19
# baseline (speedup 1.0000x reference)
"""Trainium2 Bass kernel for nn_MultiHeadAttention (B=4, S=2048, D=768, H=12).

Sharding: query-parallel. 8 cores = 4 batches x 2 query-halves. Each core
computes full K/V projections for its batch (duplicated across the 2 cores
sharing a batch) plus Q projection / attention / output projection / LayerNorm
for its 1024 query rows. No collectives needed: each core owns a disjoint
slice of the output.

On-chip layout (per core):
  qhat  [128, 6, 1024]  q~^T   bf16 (col-chunk partitions, rows free)
  kbuf  [128, 2048]     k~^T   bf16 (rotating, per 2-head chunk)
  vhat  [128, 16, 12*65] v~    bf16 (key-chunk partitions, per head 64 cols + ones)
  keep  [128, 16, 1024]  1-mask bf16
Attention per head h, key-chunk kc:
  S^T = k~_h^T.T @ q~_h^T           (PE, K=64)           -> PSUM [128,1024]
  P   = exp(S^T/8)                  (ACT, ->bf16 SBUF)
  P  *= keep[kc]                    (DVE)
  ctx^T += [v~_h | 1].T @ P         (PE, accum 16 chunks) -> PSUM [65,1024]
rowsum (partition 64) -> reciprocal -> DRAM bounce -> broadcast [64,1024]
ctx^T * recip -> ctxT bf16; out-proj + residual + LayerNorm per 128-row chunk.
"""

import sys

for _p in ("/opt/trn_rl_repo", "/root/.axon_site/_ro/trn_rl_repo"):
    if _p not in sys.path:
        sys.path.insert(0, _p)

import numpy as np
import ml_dtypes

B = 4
S = 2048
D = 768
H = 12
DK = 64
NCORES = 8
ROWS = S // 2          # 1024 query rows per core
P = 128
KO = D // P            # 6 contraction chunks
MC = D // P            # 6 output-column chunks (2 heads each)
KC = S // P            # 16 key chunks
RC = ROWS // P         # 8 row chunks
VW = DK + 1            # 65: v columns + ones column
EPS = 1e-5

BF16 = ml_dtypes.bfloat16

_cached = {}

LDW_OPT = False  # walrus ldw-opt is incompatible with bacc's InstLdweights


def _enable_ldw_opt():
    if _cached.get("ldw_patched"):
        return
    import concourse.bass_utils as bu

    orig = bu.run_command

    def patched(argv, **kwargs):
        argv = ["--enable-ldw-opt=true" if a == "--enable-ldw-opt=false" else a
                for a in argv]
        return orig(argv, **kwargs)

    bu.run_command = patched
    _cached["ldw_patched"] = True


def _build():
    import concourse.bass as bass
    import concourse.tile as tile
    import concourse.mybir as mybir
    from concourse import bacc
    from concourse import masks

    f32 = mybir.dt.float32
    bf = mybir.dt.bfloat16
    AF = mybir.ActivationFunctionType
    OP = mybir.AluOpType

    if LDW_OPT:
        _enable_ldw_opt()

    nc = bacc.Bacc("TRN2", target_bir_lowering=False, debug=False)

    NSPLIT = 512  # PSUM bank = 512 f32; matmul out must stay in one bank

    def mm_acc(ps, n_total, lhsT_fn, rhs_fn, nk):
        """ps[:, :n_total] += sum_k lhsT_k.T @ rhs_k, split into <=512-col chunks.

        k outer / n inner keeps same-weight matmuls adjacent so walrus
        ldw-opt can elide the repeated LDWEIGHTS."""
        for k in range(nk):
            lhsT = lhsT_fn(k)
            for n0 in range(0, n_total, NSPLIT):
                n1 = min(n0 + NSPLIT, n_total)
                nc.tensor.matmul(ps[:, n0:n1], lhsT, rhs_fn(k)[:, n0:n1],
                                 start=(k == 0), stop=(k == nk - 1))

    def mm_acc2(ps, n_total, lhsT_fn, rhs_fn, nk):
        """Like mm_acc but accumulates onto an already-started psum."""
        for k in range(nk):
            lhsT = lhsT_fn(k)
            for n0 in range(0, n_total, NSPLIT):
                n1 = min(n0 + NSPLIT, n_total)
                nc.tensor.matmul(ps[:, n0:n1], lhsT, rhs_fn(k)[:, n0:n1],
                                 start=False, stop=(k == nk - 1))

    qt_d = nc.dram_tensor("qt", [D, ROWS], bf, kind="ExternalInput")
    kt_d = nc.dram_tensor("kt", [D, S], bf, kind="ExternalInput")
    vt_d = nc.dram_tensor("vt", [D, S], bf, kind="ExternalInput")
    keep_d = nc.dram_tensor("keep", [S, ROWS], bf, kind="ExternalInput")
    qres_d = nc.dram_tensor("qres", [ROWS, D], f32, kind="ExternalInput")
    w_d = {n: nc.dram_tensor(n, [D, D], bf, kind="ExternalInput")
           for n in ("wq", "wk", "wv", "wo")}
    b_d = {n: nc.dram_tensor(n, [D], f32, kind="ExternalInput")
           for n in ("bq", "bk", "bv", "gamma", "beta")}
    out_d = nc.dram_tensor("out", [ROWS, D], f32, kind="ExternalOutput")

    rs_d = [nc.dram_tensor(f"rs_bounce{h}", [ROWS], f32, kind="Internal")
            for h in range(H)]
    rs2_d = [nc.dram_tensor(f"rs2_bounce{h}", [ROWS], bf, kind="Internal")
             for h in range(H)]

    def bcast_ap(handle, n):
        ap = handle.ap()
        return bass.AP(tensor=ap.tensor, offset=0, ap=[[0, P], [1, n]])

    with tile.TileContext(nc) as tc:
        with tc.tile_pool(name="wp", bufs=1) as wp, \
             tc.tile_pool(name="xin", bufs=2) as xin, \
             tc.tile_pool(name="kp", bufs=2) as kp, \
             tc.tile_pool(name="ktp", bufs=2) as ktp, \
             tc.tile_pool(name="big", bufs=1) as big, \
             tc.tile_pool(name="pp", bufs=6) as ppool, \
             tc.tile_pool(name="small", bufs=2) as small, \
             tc.tile_pool(name="ph3", bufs=2) as ph3, \
             tc.tile_pool(name="mm", bufs=2, space="PSUM") as mm, \
             tc.tile_pool(name="cx", bufs=2, space="PSUM") as cx:

            # ---- weights (DMA issue order = arrival order; wq first) ----
            w_sb = {}

            def load_w(n):
                tag = "wqo" if n in ("wq", "wo") else n
                t = wp.tile([P, KO, D], bf, tag=tag)
                src = w_d[n].ap().rearrange("(o p) n -> p o n", p=P)
                for ko in range(KO):
                    nc.sync.dma_start(out=t[:, ko, :], in_=src[:, ko, :])
                w_sb[n] = t

            bq_sb = wp.tile([P, MC], f32, tag="bq")
            bk_sb = wp.tile([P, MC], f32, tag="bk")
            nc.sync.dma_start(out=bq_sb, in_=b_d["bq"].ap().rearrange("(o p) -> p o", p=P))
            nc.sync.dma_start(out=bk_sb, in_=b_d["bk"].ap().rearrange("(o p) -> p o", p=P))
            eps_t = wp.tile([P, 1], f32, tag="eps")
            nc.vector.memset(eps_t, EPS)

            # ---- phase 1a: q~^T (inputs first so PE starts ASAP) ----
            wq_t = wp.tile([P, KO, D], bf, tag="wqo")
            wq_src = w_d["wq"].ap().rearrange("(o p) n -> p o n", p=P)
            w_sb["wq"] = wq_t
            qt_sb = xin.tile([P, KO, ROWS], bf, tag="xin")
            qt_src = qt_d.ap().rearrange("(o p) r -> p o r", p=P)
            for ko in range(KO):
                nc.sync.dma_start(out=wq_t[:, ko, :], in_=wq_src[:, ko, :])
                nc.sync.dma_start(out=qt_sb[:, ko, :], in_=qt_src[:, ko, :])

            load_w("wk")
            kt_sb = []
            kt_src = kt_d.ap().rearrange("(o p) s -> p o s", p=P)
            for half in range(2):
                t = ktp.tile([P, KO, ROWS], bf, tag="kt")
                for ko in range(KO):
                    nc.sync.dma_start(
                        out=t[:, ko, :],
                        in_=kt_src[:, ko, half * ROWS:(half + 1) * ROWS])
                kt_sb.append(t)

            keep_sb = big.tile([P, KC, ROWS], bf, tag="keep")
            keep_src = keep_d.ap().rearrange("(c p) r -> p c r", p=P)
            for c in range(4):
                nc.sync.dma_start(out=keep_sb[:, c, :], in_=keep_src[:, c, :])

            load_w("wv")
            bvb = wp.tile([P, D], bf, tag="bvb")
            gb = wp.tile([P, D], bf, tag="gb")
            bb = wp.tile([P, D], bf, tag="bb")
            nc.gpsimd.dma_start(out=bvb, in_=bcast_ap(b_d["bv"], D))
            nc.gpsimd.dma_start(out=gb, in_=bcast_ap(b_d["gamma"], D))
            nc.gpsimd.dma_start(out=bb, in_=bcast_ap(b_d["beta"], D))
            qhat = big.tile([P, MC, ROWS], bf, tag="qhat")
            for mc in range(MC):
                ps = mm.tile([P, ROWS], mybir.dt.float32, tag="mm")
                mm_acc(ps, ROWS,
                       lambda ko, mc=mc: w_sb["wq"][:, ko, mc * P:(mc + 1) * P],
                       lambda ko: qt_sb[:, ko, :], KO)
                nc.scalar.activation(out=qhat[:, mc, :], in_=ps, func=AF.Identity,
                                     bias=bq_sb[:, mc:mc + 1], scale=1.0)

            # ---- phase 1b: v~ (keys on partitions) ----
            vhat = big.tile([P, KC, H * VW], bf, tag="vhat")
            nc.vector.memset(
                vhat.rearrange("p c (h w) -> p c h w", w=VW)[:, :, :, DK:DK + 1], 1.0)
            vt_sb = []
            vt_src = vt_d.ap().rearrange("(o p) s -> p o s", p=P)
            for half in range(2):
                t = xin.tile([P, KO, ROWS], bf, tag="xin")
                for ko in range(KO):
                    nc.sync.dma_start(
                        out=t[:, ko, :],
                        in_=vt_src[:, ko, half * ROWS:(half + 1) * ROWS])
                vt_sb.append(t)
            def vproj(kc):
                half, c = divmod(kc, KC // 2)
                ps = mm.tile([P, D], mybir.dt.float32, tag="mm")
                mm_acc(ps, D,
                       lambda ko, half=half, c=c: vt_sb[half][:, ko, c * P:(c + 1) * P],
                       lambda ko: w_sb["wv"][:, ko, :], KO)
                dst = vhat.rearrange("p c (h w) -> p c h w", w=VW)[:, kc, :, 0:DK]
                nc.vector.tensor_tensor(
                    out=dst, in0=ps.rearrange("p (h w) -> p h w", w=DK),
                    in1=bvb.rearrange("p (h w) -> p h w", w=DK), op=OP.add)

            # ---- phase 1c + 2: k~ per 2-head chunk, then attention ----
            for c in range(4, KC):
                nc.sync.dma_start(out=keep_sb[:, c, :], in_=keep_src[:, c, :])
            load_w("wo")

            ctxT = big.tile([P, MC, ROWS], bf, tag="ctxT")
            vh4 = vhat.rearrange("p c (h w) -> p c h w", w=VW)

            for mc in range(MC):
                kbuf = kp.tile([P, S], bf, tag="kbuf")
                for half in range(2):
                    ps = mm.tile([P, ROWS], mybir.dt.float32, tag="mm")
                    mm_acc(ps, ROWS,
                           lambda ko, mc=mc: w_sb["wk"][:, ko, mc * P:(mc + 1) * P],
                           lambda ko, half=half: kt_sb[half][:, ko, :], KO)
                    nc.scalar.activation(out=kbuf[:, half * ROWS:(half + 1) * ROWS],
                                         in_=ps, func=AF.Identity,
                                         bias=bk_sb[:, mc:mc + 1], scale=1.0)
                ctx_pair = [cx.tile([VW, ROWS], mybir.dt.float32, tag="cx",
                                    name=f"ctx_ps_{mc}_{i}")
                            for i in range(2)]
                for kc in range(KC):
                    if mc == 0:
                        vproj(kc)
                    for hh in range(2):
                        h = 2 * mc + hh
                        pr = slice(hh * DK, (hh + 1) * DK)
                        s_ps = mm.tile([P, ROWS], mybir.dt.float32, tag="mm")
                        for n0 in range(0, ROWS, NSPLIT):
                            nc.tensor.matmul(s_ps[:, n0:n0 + NSPLIT],
                                             kbuf[pr, kc * P:(kc + 1) * P],
                                             qhat[pr, mc, n0:n0 + NSPLIT],
                                             start=True, stop=True)
                        p_t = ppool.tile([P, ROWS], bf, tag="p")
                        nc.scalar.activation(out=p_t, in_=s_ps, func=AF.Exp,
                                             scale=1.0 / np.sqrt(DK))
                        nc.vector.tensor_tensor(out=p_t, in0=p_t,
                                                in1=keep_sb[:, kc, :], op=OP.mult)
                        for n0 in range(0, ROWS, NSPLIT):
                            nc.tensor.matmul(ctx_pair[hh][:, n0:n0 + NSPLIT],
                                             vh4[:, kc, h, :], p_t[:, n0:n0 + NSPLIT],
                                             start=(kc == 0), stop=(kc == KC - 1))
                for hh in range(2):
                    h = 2 * mc + hh
                    pr = slice(hh * DK, (hh + 1) * DK)
                    ctx_ps = ctx_pair[hh]
                    # rowsum reciprocal: extract -> DRAM -> [128,8] transpose ->
                    # cheap reciprocal -> DRAM -> broadcast to 64 partitions.
                    rs_t = small.tile([1, ROWS], f32, tag="rs")
                    nc.vector.tensor_copy(out=rs_t, in_=ctx_ps[DK:DK + 1, :])
                    nc.sync.dma_start(out=rs_d[h].ap(), in_=rs_t)
                    rsT = small.tile([P, ROWS // P], f32, tag="rsT")
                    nc.sync.dma_start(
                        out=rsT, in_=rs_d[h].ap().rearrange("(p o) -> p o", p=P))
                    rsTb = small.tile([P, ROWS // P], bf, tag="rsTb")
                    with nc.allow_low_precision(reason="softmax rowsum recip in bf16"):
                        nc.vector.reciprocal(out=rsTb, in_=rsT)
                    nc.sync.dma_start(
                        out=rs2_d[h].ap().rearrange("(p o) -> p o", p=P), in_=rsTb)
                    rsb_t = small.tile([DK, ROWS], bf, tag="rsb")
                    nc.sync.dma_start(
                        out=rsb_t,
                        in_=bass.AP(tensor=rs2_d[h].ap().tensor, offset=0,
                                    ap=[[0, DK], [1, ROWS]]))
                    nc.vector.tensor_tensor(out=ctxT[pr, mc, :],
                                            in0=ctx_ps[0:DK, :], in1=rsb_t,
                                            op=OP.mult)

            # ---- phase 3: out projection + residual + LayerNorm ----
            nsub = 3
            sub = D // nsub  # 256 <= BN_STATS_FMAX
            for rc in range(RC):
                ps = mm.tile([P, D], mybir.dt.float32, tag="mm")
                mm_acc(ps, D,
                       lambda ko, rc=rc: ctxT[:, ko, rc * P:(rc + 1) * P],
                       lambda ko: w_sb["wo"][:, ko, :], KO)
                qres_t = ph3.tile([P, D], f32, tag="qres")
                nc.sync.dma_start(out=qres_t,
                                  in_=qres_d.ap()[rc * P:(rc + 1) * P, :])
                x_t = ph3.tile([P, D], f32, tag="x")
                nc.vector.tensor_tensor(out=x_t, in0=ps, in1=qres_t, op=OP.add)
                stats = small.tile([P, nsub, 6], f32, tag="stats")
                for sg in range(nsub):
                    nc.vector.bn_stats(out=stats[:, sg, :],
                                       in_=x_t[:, sg * sub:(sg + 1) * sub])
                mv = small.tile([P, 2], f32, tag="mv")
                nc.vector.bn_aggr(out=mv, in_=stats)
                std_t = small.tile([P, 1], f32, tag="std")
                nc.scalar.activation(out=std_t, in_=mv[:, 1:2], func=AF.Sqrt,
                                     bias=eps_t, scale=1.0)
                nc.vector.reciprocal(out=std_t, in_=std_t)
                nmr = small.tile([P, 1], f32, tag="nmr")
                nc.vector.scalar_tensor_tensor(out=nmr, in0=mv[:, 0:1], scalar=-1.0,
                                               in1=std_t, op0=OP.mult, op1=OP.mult)
                nc.scalar.activation(out=x_t, in_=x_t, func=AF.Identity,
                                     bias=nmr, scale=std_t)
                nc.vector.tensor_tensor(out=x_t, in0=x_t, in1=gb, op=OP.mult)
                nc.vector.tensor_tensor(out=x_t, in0=x_t, in1=bb, op=OP.add)
                nc.sync.dma_start(out=out_d.ap()[rc * P:(rc + 1) * P, :], in_=x_t)

    nc.compile()
    return nc


def _get_nc():
    if "nc" not in _cached:
        _cached["nc"] = _build()
    return _cached["nc"]


def _make_in_maps(Q, Kt, V, attn_mask, Wq, bq, Wk, bk, Wv, bv, Wo, bo, gamma, beta):
    f32 = np.float32
    w = {"wq": np.ascontiguousarray(Wq, f32).astype(BF16),
         "wk": np.ascontiguousarray(Wk, f32).astype(BF16),
         "wv": np.ascontiguousarray(Wv, f32).astype(BF16),
         "wo": np.ascontiguousarray(Wo, f32).astype(BF16)}
    b = {"bq": np.ascontiguousarray(bq, f32), "bk": np.ascontiguousarray(bk, f32),
         "bv": np.ascontiguousarray(bv, f32),
         "gamma": np.ascontiguousarray(gamma, f32),
         "beta": np.ascontiguousarray(beta, f32)}
    bo_f = np.asarray(bo, f32)
    in_maps = []
    for c in range(NCORES):
        bidx, half = divmod(c, 2)
        rows = slice(half * ROWS, (half + 1) * ROWS)
        m = {
            "qt": np.ascontiguousarray(Q[bidx, rows].T).astype(BF16),
            "kt": np.ascontiguousarray(Kt[bidx].T).astype(BF16),
            "vt": np.ascontiguousarray(V[bidx].T).astype(BF16),
            "keep": np.ascontiguousarray(
                (~attn_mask[bidx, rows]).T.astype(BF16)),
            "qres": np.ascontiguousarray(Q[bidx, rows], f32) + bo_f,
        }
        m.update(w)
        m.update(b)
        in_maps.append(m)
    return in_maps


def kernel(Q, K, V, attn_mask, Wq, bq, Wk, bk, Wv, bv, Wo, bo, gamma, beta,
           _profile=None):
    from concourse.bass_utils import run_bass_kernel_spmd

    nc = _get_nc()
    in_maps = _make_in_maps(np.asarray(Q, np.float32), np.asarray(K, np.float32),
                            np.asarray(V, np.float32), np.asarray(attn_mask),
                            Wq, bq, Wk, bk, Wv, bv, Wo, bo, gamma, beta)
    kwargs = dict(_profile) if _profile else {}
    res = run_bass_kernel_spmd(nc, in_maps, list(range(NCORES)), **kwargs)
    if _profile is not None:
        _cached["last_results"] = res
    out = np.empty((B, S, D), np.float32)
    for c, m in enumerate(res.results):
        bidx, half = divmod(c, 2)
        out[bidx, half * ROWS:(half + 1) * ROWS] = m["out"]
    return out


# revision 20
# speedup vs baseline: 1.1383x; 1.1383x over previous
"""Trainium2 Bass kernel for nn_MultiHeadAttention (B=4, S=2048, D=768, H=12).

Sharding: query-parallel. 8 cores = 4 batches x 2 query-halves. Each core
computes full K/V projections for its batch (duplicated across the 2 cores
sharing a batch) plus Q projection / attention / output projection / LayerNorm
for its 1024 query rows. No collectives needed: each core owns a disjoint
slice of the output.

On-chip layout (per core):
  qhat  [128, 6, 1024]  q~^T   bf16 (col-chunk partitions, rows free)
  kbuf  [128, 2048]     k~^T   bf16 (rotating, per 2-head chunk)
  vhat  [128, 16, 12*65] v~    bf16 (key-chunk partitions, per head 64 cols + ones)
  keep  [128, 16, 1024]  1-mask bf16
Attention per head h, key-chunk kc:
  S^T = k~_h^T.T @ q~_h^T           (PE, K=64)           -> PSUM [128,1024]
  P   = exp(S^T/8)                  (ACT, ->bf16 SBUF)
  P  *= keep[kc]                    (DVE)
  ctx^T += [v~_h | 1].T @ P         (PE, accum 16 chunks) -> PSUM [65,1024]
rowsum (partition 64) -> reciprocal -> DRAM bounce -> broadcast [64,1024]
ctx^T * recip -> ctxT bf16; out-proj + residual + LayerNorm per 128-row chunk.
"""

import sys

for _p in ("/opt/trn_rl_repo", "/root/.axon_site/_ro/trn_rl_repo"):
    if _p not in sys.path:
        sys.path.insert(0, _p)

import numpy as np
import ml_dtypes

B = 4
S = 2048
D = 768
H = 12
DK = 64
NCORES = 8
ROWS = S // 2          # 1024 query rows per core
P = 128
KO = D // P            # 6 contraction chunks
MC = D // P            # 6 output-column chunks (2 heads each)
KC = S // P            # 16 key chunks
RC = ROWS // P         # 8 row chunks
VW = DK + 1            # 65: v columns + ones column
EPS = 1e-5

BF16 = ml_dtypes.bfloat16

_cached = {}

LDW_OPT = False  # walrus ldw-opt is incompatible with bacc's InstLdweights


def _enable_ldw_opt():
    if _cached.get("ldw_patched"):
        return
    import concourse.bass_utils as bu

    orig = bu.run_command

    def patched(argv, **kwargs):
        argv = ["--enable-ldw-opt=true" if a == "--enable-ldw-opt=false" else a
                for a in argv]
        return orig(argv, **kwargs)

    bu.run_command = patched
    _cached["ldw_patched"] = True


def _build():
    import concourse.bass as bass
    import concourse.tile as tile
    import concourse.mybir as mybir
    from concourse import bacc
    from concourse import masks

    f32 = mybir.dt.float32
    bf = mybir.dt.bfloat16
    AF = mybir.ActivationFunctionType
    OP = mybir.AluOpType

    if LDW_OPT:
        _enable_ldw_opt()

    nc = bacc.Bacc("TRN2", target_bir_lowering=False, debug=False)

    NSPLIT = 512  # PSUM bank = 512 f32; matmul out must stay in one bank

    def mm_acc(ps, n_total, lhsT_fn, rhs_fn, nk):
        """ps[:, :n_total] += sum_k lhsT_k.T @ rhs_k, split into <=512-col chunks.

        k outer / n inner keeps same-weight matmuls adjacent so walrus
        ldw-opt can elide the repeated LDWEIGHTS."""
        for k in range(nk):
            lhsT = lhsT_fn(k)
            for n0 in range(0, n_total, NSPLIT):
                n1 = min(n0 + NSPLIT, n_total)
                nc.tensor.matmul(ps[:, n0:n1], lhsT, rhs_fn(k)[:, n0:n1],
                                 start=(k == 0), stop=(k == nk - 1))

    def mm_acc2(ps, n_total, lhsT_fn, rhs_fn, nk):
        """Like mm_acc but accumulates onto an already-started psum."""
        for k in range(nk):
            lhsT = lhsT_fn(k)
            for n0 in range(0, n_total, NSPLIT):
                n1 = min(n0 + NSPLIT, n_total)
                nc.tensor.matmul(ps[:, n0:n1], lhsT, rhs_fn(k)[:, n0:n1],
                                 start=False, stop=(k == nk - 1))

    qt_d = nc.dram_tensor("qt", [D, ROWS], bf, kind="ExternalInput")
    kt_d = nc.dram_tensor("kt", [D, S], bf, kind="ExternalInput")
    vt_d = nc.dram_tensor("vt", [D, S], bf, kind="ExternalInput")
    keep_d = nc.dram_tensor("keep", [S, ROWS], bf, kind="ExternalInput")
    qres_d = nc.dram_tensor("qres", [ROWS, D], f32, kind="ExternalInput")
    w_d = {n: nc.dram_tensor(n, [D, D], bf, kind="ExternalInput")
           for n in ("wq", "wk", "wv", "wo")}
    b_d = {n: nc.dram_tensor(n, [D], f32, kind="ExternalInput")
           for n in ("bq", "bk", "bv", "gamma", "beta")}
    out_d = nc.dram_tensor("out", [ROWS, D], f32, kind="ExternalOutput")

    rs_d = [nc.dram_tensor(f"rs_bounce{h}", [ROWS], f32, kind="Internal")
            for h in range(H)]
    rs2_d = [nc.dram_tensor(f"rs2_bounce{h}", [ROWS], bf, kind="Internal")
             for h in range(H)]

    def bcast_ap(handle, n):
        ap = handle.ap()
        return bass.AP(tensor=ap.tensor, offset=0, ap=[[0, P], [1, n]])

    with tile.TileContext(nc) as tc:
        with tc.tile_pool(name="wp", bufs=1) as wp, \
             tc.tile_pool(name="xin", bufs=2) as xin, \
             tc.tile_pool(name="kp", bufs=2) as kp, \
             tc.tile_pool(name="ktp", bufs=2) as ktp, \
             tc.tile_pool(name="big", bufs=1) as big, \
             tc.tile_pool(name="pp", bufs=4) as ppool, \
             tc.tile_pool(name="small", bufs=2) as small, \
             tc.tile_pool(name="ph3", bufs=2) as ph3, \
             tc.tile_pool(name="mm", bufs=2, space="PSUM") as mm, \
             tc.tile_pool(name="cx", bufs=2, space="PSUM") as cx:

            # ---- weights (DMA issue order = arrival order; wq first) ----
            w_sb = {}

            def load_w(n):
                tag = "wqo" if n in ("wq", "wo") else n
                t = wp.tile([P, KO, D], bf, tag=tag)
                src = w_d[n].ap().rearrange("(o p) n -> p o n", p=P)
                for ko in range(KO):
                    nc.sync.dma_start(out=t[:, ko, :], in_=src[:, ko, :])
                w_sb[n] = t

            bq_sb = wp.tile([P, MC], f32, tag="bq")
            bk_sb = wp.tile([P, MC], f32, tag="bk")
            nc.sync.dma_start(out=bq_sb, in_=b_d["bq"].ap().rearrange("(o p) -> p o", p=P))
            nc.sync.dma_start(out=bk_sb, in_=b_d["bk"].ap().rearrange("(o p) -> p o", p=P))
            eps_t = wp.tile([P, 1], f32, tag="eps")
            nc.vector.memset(eps_t, EPS)

            # ---- phase 1a: q~^T (inputs first so PE starts ASAP) ----
            wq_t = wp.tile([P, KO, D], bf, tag="wqo")
            wq_src = w_d["wq"].ap().rearrange("(o p) n -> p o n", p=P)
            w_sb["wq"] = wq_t
            qt_sb = xin.tile([P, KO, ROWS], bf, tag="xin")
            qt_src = qt_d.ap().rearrange("(o p) r -> p o r", p=P)
            for ko in range(KO):
                nc.sync.dma_start(out=wq_t[:, ko, :], in_=wq_src[:, ko, :])
                nc.sync.dma_start(out=qt_sb[:, ko, :], in_=qt_src[:, ko, :])

            load_w("wk")
            kt_sb = []
            kt_src = kt_d.ap().rearrange("(o p) s -> p o s", p=P)
            for half in range(2):
                t = ktp.tile([P, KO, ROWS], bf, tag="kt")
                for ko in range(KO):
                    nc.sync.dma_start(
                        out=t[:, ko, :],
                        in_=kt_src[:, ko, half * ROWS:(half + 1) * ROWS])
                kt_sb.append(t)

            keep_sb = big.tile([P, KC, ROWS], bf, tag="keep")
            keep_src = keep_d.ap().rearrange("(c p) r -> p c r", p=P)
            for c in range(4):
                nc.sync.dma_start(out=keep_sb[:, c, :], in_=keep_src[:, c, :])

            load_w("wv")
            bvb = wp.tile([P, D], bf, tag="bvb")
            gb = wp.tile([P, D], bf, tag="gb")
            bb = wp.tile([P, D], bf, tag="bb")
            nc.gpsimd.dma_start(out=bvb, in_=bcast_ap(b_d["bv"], D))
            nc.gpsimd.dma_start(out=gb, in_=bcast_ap(b_d["gamma"], D))
            nc.gpsimd.dma_start(out=bb, in_=bcast_ap(b_d["beta"], D))
            qhat = big.tile([P, MC, ROWS], bf, tag="qhat")
            for mc in range(MC):
                ps = mm.tile([P, ROWS], mybir.dt.float32, tag="mm")
                mm_acc(ps, ROWS,
                       lambda ko, mc=mc: w_sb["wq"][:, ko, mc * P:(mc + 1) * P],
                       lambda ko: qt_sb[:, ko, :], KO)
                nc.scalar.activation(out=qhat[:, mc, :], in_=ps, func=AF.Identity,
                                     bias=bq_sb[:, mc:mc + 1], scale=1.0)

            # ---- phase 1b: v~ (keys on partitions) ----
            vhat = big.tile([P, KC, H * VW], bf, tag="vhat")
            nc.vector.memset(
                vhat.rearrange("p c (h w) -> p c h w", w=VW)[:, :, :, DK:DK + 1], 1.0)
            vt_sb = []
            vt_src = vt_d.ap().rearrange("(o p) s -> p o s", p=P)
            for half in range(2):
                t = xin.tile([P, KO, ROWS], bf, tag="xin")
                for ko in range(KO):
                    nc.sync.dma_start(
                        out=t[:, ko, :],
                        in_=vt_src[:, ko, half * ROWS:(half + 1) * ROWS])
                vt_sb.append(t)
            def vproj(kc):
                half, c = divmod(kc, KC // 2)
                ps = mm.tile([P, D], mybir.dt.float32, tag="mm")
                mm_acc(ps, D,
                       lambda ko, half=half, c=c: vt_sb[half][:, ko, c * P:(c + 1) * P],
                       lambda ko: w_sb["wv"][:, ko, :], KO)
                dst = vhat.rearrange("p c (h w) -> p c h w", w=VW)[:, kc, :, 0:DK]
                nc.vector.tensor_tensor(
                    out=dst, in0=ps.rearrange("p (h w) -> p h w", w=DK),
                    in1=bvb.rearrange("p (h w) -> p h w", w=DK), op=OP.add)

            # ---- phase 1c + 2: k~ per 2-head chunk, then attention ----
            for c in range(4, KC):
                nc.sync.dma_start(out=keep_sb[:, c, :], in_=keep_src[:, c, :])
            load_w("wo")

            ctxT = big.tile([P, MC, ROWS], bf, tag="ctxT")
            vh4 = vhat.rearrange("p c (h w) -> p c h w", w=VW)

            for mc in range(MC):
                kbuf = kp.tile([P, S], bf, tag="kbuf")
                for half in range(2):
                    ps = mm.tile([P, ROWS], mybir.dt.float32, tag="mm")
                    mm_acc(ps, ROWS,
                           lambda ko, mc=mc: w_sb["wk"][:, ko, mc * P:(mc + 1) * P],
                           lambda ko, half=half: kt_sb[half][:, ko, :], KO)
                    nc.scalar.activation(out=kbuf[:, half * ROWS:(half + 1) * ROWS],
                                         in_=ps, func=AF.Identity,
                                         bias=bk_sb[:, mc:mc + 1], scale=1.0)
                for hh in range(2):
                    h = 2 * mc + hh
                    pr = slice(hh * DK, (hh + 1) * DK)
                    ctx_ps = cx.tile([VW, ROWS], mybir.dt.float32, tag="cx")
                    for kc in range(KC):
                        if h == 0:
                            vproj(kc)
                        s_ps = mm.tile([P, ROWS], mybir.dt.float32, tag="mm")
                        for n0 in range(0, ROWS, NSPLIT):
                            nc.tensor.matmul(s_ps[:, n0:n0 + NSPLIT],
                                             kbuf[pr, kc * P:(kc + 1) * P],
                                             qhat[pr, mc, n0:n0 + NSPLIT],
                                             start=True, stop=True)
                        p_t = ppool.tile([P, ROWS], bf, tag="p")
                        nc.scalar.activation(out=p_t, in_=s_ps, func=AF.Exp,
                                             scale=1.0 / np.sqrt(DK))
                        nc.vector.tensor_tensor(out=p_t, in0=p_t,
                                                in1=keep_sb[:, kc, :], op=OP.mult)
                        for n0 in range(0, ROWS, NSPLIT):
                            nc.tensor.matmul(ctx_ps[:, n0:n0 + NSPLIT],
                                             vh4[:, kc, h, :], p_t[:, n0:n0 + NSPLIT],
                                             start=(kc == 0), stop=(kc == KC - 1))
                    # rowsum reciprocal: extract -> DRAM -> [128,8] transpose ->
                    # cheap reciprocal -> DRAM -> broadcast to 64 partitions.
                    rs_t = small.tile([1, ROWS], f32, tag="rs")
                    nc.vector.tensor_copy(out=rs_t, in_=ctx_ps[DK:DK + 1, :])
                    nc.sync.dma_start(out=rs_d[h].ap(), in_=rs_t)
                    rsT = small.tile([P, ROWS // P], f32, tag="rsT")
                    nc.sync.dma_start(
                        out=rsT, in_=rs_d[h].ap().rearrange("(p o) -> p o", p=P))
                    rsTb = small.tile([P, ROWS // P], bf, tag="rsTb")
                    with nc.allow_low_precision(reason="softmax rowsum recip in bf16"):
                        nc.vector.reciprocal(out=rsTb, in_=rsT)
                    nc.sync.dma_start(
                        out=rs2_d[h].ap().rearrange("(p o) -> p o", p=P), in_=rsTb)
                    rsb_t = small.tile([DK, ROWS], bf, tag="rsb")
                    nc.sync.dma_start(
                        out=rsb_t,
                        in_=bass.AP(tensor=rs2_d[h].ap().tensor, offset=0,
                                    ap=[[0, DK], [1, ROWS]]))
                    nc.vector.tensor_tensor(out=ctxT[pr, mc, :],
                                            in0=ctx_ps[0:DK, :], in1=rsb_t,
                                            op=OP.mult)

            # ---- phase 3: out projection + residual + LayerNorm ----
            nsub = 3
            sub = D // nsub  # 256 <= BN_STATS_FMAX
            for rc in range(RC):
                ps = mm.tile([P, D], mybir.dt.float32, tag="mm")
                mm_acc(ps, D,
                       lambda ko, rc=rc: ctxT[:, ko, rc * P:(rc + 1) * P],
                       lambda ko: w_sb["wo"][:, ko, :], KO)
                qres_t = ph3.tile([P, D], f32, tag="qres")
                nc.sync.dma_start(out=qres_t,
                                  in_=qres_d.ap()[rc * P:(rc + 1) * P, :])
                x_t = ph3.tile([P, D], f32, tag="x")
                nc.vector.tensor_tensor(out=x_t, in0=ps, in1=qres_t, op=OP.add)
                stats = small.tile([P, nsub, 6], f32, tag="stats")
                for sg in range(nsub):
                    nc.vector.bn_stats(out=stats[:, sg, :],
                                       in_=x_t[:, sg * sub:(sg + 1) * sub])
                mv = small.tile([P, 2], f32, tag="mv")
                nc.vector.bn_aggr(out=mv, in_=stats)
                std_t = small.tile([P, 1], f32, tag="std")
                nc.scalar.activation(out=std_t, in_=mv[:, 1:2], func=AF.Sqrt,
                                     bias=eps_t, scale=1.0)
                nc.vector.reciprocal(out=std_t, in_=std_t)
                nc.vector.tensor_scalar(out=x_t, in0=x_t, scalar1=mv[:, 0:1],
                                        scalar2=std_t, op0=OP.subtract,
                                        op1=OP.mult)
                nc.vector.tensor_tensor(out=x_t, in0=x_t, in1=gb, op=OP.mult)
                nc.vector.tensor_tensor(out=x_t, in0=x_t, in1=bb, op=OP.add)
                nc.sync.dma_start(out=out_d.ap()[rc * P:(rc + 1) * P, :], in_=x_t)

    nc.compile()
    return nc


def _get_nc():
    if "nc" not in _cached:
        _cached["nc"] = _build()
    return _cached["nc"]


def _make_in_maps(Q, Kt, V, attn_mask, Wq, bq, Wk, bk, Wv, bv, Wo, bo, gamma, beta):
    f32 = np.float32
    w = {"wq": np.ascontiguousarray(Wq, f32).astype(BF16),
         "wk": np.ascontiguousarray(Wk, f32).astype(BF16),
         "wv": np.ascontiguousarray(Wv, f32).astype(BF16),
         "wo": np.ascontiguousarray(Wo, f32).astype(BF16)}
    b = {"bq": np.ascontiguousarray(bq, f32), "bk": np.ascontiguousarray(bk, f32),
         "bv": np.ascontiguousarray(bv, f32),
         "gamma": np.ascontiguousarray(gamma, f32),
         "beta": np.ascontiguousarray(beta, f32)}
    bo_f = np.asarray(bo, f32)
    in_maps = []
    for c in range(NCORES):
        bidx, half = divmod(c, 2)
        rows = slice(half * ROWS, (half + 1) * ROWS)
        m = {
            "qt": np.ascontiguousarray(Q[bidx, rows].T).astype(BF16),
            "kt": np.ascontiguousarray(Kt[bidx].T).astype(BF16),
            "vt": np.ascontiguousarray(V[bidx].T).astype(BF16),
            "keep": np.ascontiguousarray(
                (~attn_mask[bidx, rows]).T.astype(BF16)),
            "qres": np.ascontiguousarray(Q[bidx, rows], f32) + bo_f,
        }
        m.update(w)
        m.update(b)
        in_maps.append(m)
    return in_maps


def kernel(Q, K, V, attn_mask, Wq, bq, Wk, bk, Wv, bv, Wo, bo, gamma, beta,
           _profile=None):
    from concourse.bass_utils import run_bass_kernel_spmd

    nc = _get_nc()
    in_maps = _make_in_maps(np.asarray(Q, np.float32), np.asarray(K, np.float32),
                            np.asarray(V, np.float32), np.asarray(attn_mask),
                            Wq, bq, Wk, bk, Wv, bv, Wo, bo, gamma, beta)
    kwargs = dict(_profile) if _profile else {}
    res = run_bass_kernel_spmd(nc, in_maps, list(range(NCORES)), **kwargs)
    if _profile is not None:
        _cached["last_results"] = res
    out = np.empty((B, S, D), np.float32)
    for c, m in enumerate(res.results):
        bidx, half = divmod(c, 2)
        out[bidx, half * ROWS:(half + 1) * ROWS] = m["out"]
    return out


# revision 21
# speedup vs baseline: 1.1766x; 1.0337x over previous
"""Trainium2 Bass kernel for nn_MultiHeadAttention (B=4, S=2048, D=768, H=12).

Sharding: query-parallel. 8 cores = 4 batches x 2 query-halves. Each core
computes full K/V projections for its batch (duplicated across the 2 cores
sharing a batch) plus Q projection / attention / output projection / LayerNorm
for its 1024 query rows. No collectives needed: each core owns a disjoint
slice of the output.

On-chip layout (per core):
  qhat  [128, 6, 1024]  q~^T   bf16 (col-chunk partitions, rows free)
  kbuf  [128, 2048]     k~^T   bf16 (rotating, per 2-head chunk)
  vhat  [128, 16, 12*65] v~    bf16 (key-chunk partitions, per head 64 cols + ones)
  keep  [128, 16, 1024]  1-mask bf16
Attention per head h, key-chunk kc:
  S^T = k~_h^T.T @ q~_h^T           (PE, K=64)           -> PSUM [128,1024]
  P   = exp(S^T/8)                  (ACT, ->bf16 SBUF)
  P  *= keep[kc]                    (DVE)
  ctx^T += [v~_h | 1].T @ P         (PE, accum 16 chunks) -> PSUM [65,1024]
rowsum (partition 64) -> reciprocal -> DRAM bounce -> broadcast [64,1024]
ctx^T * recip -> ctxT bf16; out-proj + residual + LayerNorm per 128-row chunk.
"""

import sys

for _p in ("/opt/trn_rl_repo", "/root/.axon_site/_ro/trn_rl_repo"):
    if _p not in sys.path:
        sys.path.insert(0, _p)

import numpy as np
import ml_dtypes

B = 4
S = 2048
D = 768
H = 12
DK = 64
NCORES = 8
ROWS = S // 2          # 1024 query rows per core
P = 128
KO = D // P            # 6 contraction chunks
MC = D // P            # 6 output-column chunks (2 heads each)
KC = S // P            # 16 key chunks
RC = ROWS // P         # 8 row chunks
VW = DK + 1            # 65: v columns + ones column
EPS = 1e-5

BF16 = ml_dtypes.bfloat16

_cached = {}

LDW_OPT = False  # walrus ldw-opt is incompatible with bacc's InstLdweights


def _enable_ldw_opt():
    if _cached.get("ldw_patched"):
        return
    import concourse.bass_utils as bu

    orig = bu.run_command

    def patched(argv, **kwargs):
        argv = ["--enable-ldw-opt=true" if a == "--enable-ldw-opt=false" else a
                for a in argv]
        return orig(argv, **kwargs)

    bu.run_command = patched
    _cached["ldw_patched"] = True


def _build():
    import concourse.bass as bass
    import concourse.tile as tile
    import concourse.mybir as mybir
    from concourse import bacc
    from concourse import masks

    f32 = mybir.dt.float32
    bf = mybir.dt.bfloat16
    AF = mybir.ActivationFunctionType
    OP = mybir.AluOpType

    if LDW_OPT:
        _enable_ldw_opt()

    nc = bacc.Bacc("TRN2", target_bir_lowering=False, debug=False)

    NSPLIT = 512  # PSUM bank = 512 f32; matmul out must stay in one bank

    def mm_acc(ps, n_total, lhsT_fn, rhs_fn, nk):
        """ps[:, :n_total] += sum_k lhsT_k.T @ rhs_k, split into <=512-col chunks.

        k outer / n inner keeps same-weight matmuls adjacent so walrus
        ldw-opt can elide the repeated LDWEIGHTS."""
        for k in range(nk):
            lhsT = lhsT_fn(k)
            for n0 in range(0, n_total, NSPLIT):
                n1 = min(n0 + NSPLIT, n_total)
                nc.tensor.matmul(ps[:, n0:n1], lhsT, rhs_fn(k)[:, n0:n1],
                                 start=(k == 0), stop=(k == nk - 1))

    def mm_acc2(ps, n_total, lhsT_fn, rhs_fn, nk):
        """Like mm_acc but accumulates onto an already-started psum."""
        for k in range(nk):
            lhsT = lhsT_fn(k)
            for n0 in range(0, n_total, NSPLIT):
                n1 = min(n0 + NSPLIT, n_total)
                nc.tensor.matmul(ps[:, n0:n1], lhsT, rhs_fn(k)[:, n0:n1],
                                 start=False, stop=(k == nk - 1))

    qt_d = nc.dram_tensor("qt", [D, ROWS], bf, kind="ExternalInput")
    kt_d = nc.dram_tensor("kt", [D, S], bf, kind="ExternalInput")
    vt_d = nc.dram_tensor("vt", [D, S], bf, kind="ExternalInput")
    keep_d = nc.dram_tensor("keep", [S, ROWS], bf, kind="ExternalInput")
    qres_d = nc.dram_tensor("qres", [ROWS, D], f32, kind="ExternalInput")
    w_d = {n: nc.dram_tensor(n, [D, D], bf, kind="ExternalInput")
           for n in ("wq", "wk", "wv", "wo")}
    b_d = {n: nc.dram_tensor(n, [D], f32, kind="ExternalInput")
           for n in ("bq", "bk", "bv", "gamma", "beta")}
    out_d = nc.dram_tensor("out", [ROWS, D], f32, kind="ExternalOutput")

    rs_d = [nc.dram_tensor(f"rs_bounce{h}", [ROWS], f32, kind="Internal")
            for h in range(H)]
    rs2_d = [nc.dram_tensor(f"rs2_bounce{h}", [ROWS], bf, kind="Internal")
             for h in range(H)]

    def bcast_ap(handle, n):
        ap = handle.ap()
        return bass.AP(tensor=ap.tensor, offset=0, ap=[[0, P], [1, n]])

    with tile.TileContext(nc) as tc:
        with tc.tile_pool(name="wp", bufs=1) as wp, \
             tc.tile_pool(name="xin", bufs=2) as xin, \
             tc.tile_pool(name="kp", bufs=2) as kp, \
             tc.tile_pool(name="ktp", bufs=2) as ktp, \
             tc.tile_pool(name="big", bufs=1) as big, \
             tc.tile_pool(name="pp", bufs=6) as ppool, \
             tc.tile_pool(name="small", bufs=2) as small, \
             tc.tile_pool(name="ph3", bufs=2) as ph3, \
             tc.tile_pool(name="mm", bufs=2, space="PSUM") as mm, \
             tc.tile_pool(name="cx", bufs=2, space="PSUM") as cx:

            # ---- weights (DMA issue order = arrival order; wq first) ----
            w_sb = {}

            def load_w(n):
                tag = "wqo" if n in ("wq", "wo") else n
                t = wp.tile([P, KO, D], bf, tag=tag)
                src = w_d[n].ap().rearrange("(o p) n -> p o n", p=P)
                for ko in range(KO):
                    nc.sync.dma_start(out=t[:, ko, :], in_=src[:, ko, :])
                w_sb[n] = t

            bq_sb = wp.tile([P, MC], f32, tag="bq")
            bk_sb = wp.tile([P, MC], f32, tag="bk")
            nc.sync.dma_start(out=bq_sb, in_=b_d["bq"].ap().rearrange("(o p) -> p o", p=P))
            nc.sync.dma_start(out=bk_sb, in_=b_d["bk"].ap().rearrange("(o p) -> p o", p=P))
            eps_t = wp.tile([P, 1], f32, tag="eps")
            nc.vector.memset(eps_t, EPS)

            # ---- phase 1a: q~^T (inputs first so PE starts ASAP) ----
            wq_t = wp.tile([P, KO, D], bf, tag="wqo")
            wq_src = w_d["wq"].ap().rearrange("(o p) n -> p o n", p=P)
            w_sb["wq"] = wq_t
            qt_sb = xin.tile([P, KO, ROWS], bf, tag="xin")
            qt_src = qt_d.ap().rearrange("(o p) r -> p o r", p=P)
            for ko in range(KO):
                nc.sync.dma_start(out=wq_t[:, ko, :], in_=wq_src[:, ko, :])
                nc.sync.dma_start(out=qt_sb[:, ko, :], in_=qt_src[:, ko, :])

            load_w("wk")
            kt_sb = []
            kt_src = kt_d.ap().rearrange("(o p) s -> p o s", p=P)
            for half in range(2):
                t = ktp.tile([P, KO, ROWS], bf, tag="kt")
                for ko in range(KO):
                    nc.sync.dma_start(
                        out=t[:, ko, :],
                        in_=kt_src[:, ko, half * ROWS:(half + 1) * ROWS])
                kt_sb.append(t)

            keep_sb = big.tile([P, KC, ROWS], bf, tag="keep")
            keep_src = keep_d.ap().rearrange("(c p) r -> p c r", p=P)
            for c in range(4):
                nc.sync.dma_start(out=keep_sb[:, c, :], in_=keep_src[:, c, :])

            load_w("wv")
            bvb = wp.tile([P, D], bf, tag="bvb")
            gb = wp.tile([P, D], bf, tag="gb")
            bb = wp.tile([P, D], bf, tag="bb")
            nc.gpsimd.dma_start(out=bvb, in_=bcast_ap(b_d["bv"], D))
            nc.gpsimd.dma_start(out=gb, in_=bcast_ap(b_d["gamma"], D))
            nc.gpsimd.dma_start(out=bb, in_=bcast_ap(b_d["beta"], D))
            qhat = big.tile([P, MC, ROWS], bf, tag="qhat")
            for mc in range(MC):
                ps = mm.tile([P, ROWS], mybir.dt.float32, tag="mm")
                mm_acc(ps, ROWS,
                       lambda ko, mc=mc: w_sb["wq"][:, ko, mc * P:(mc + 1) * P],
                       lambda ko: qt_sb[:, ko, :], KO)
                nc.scalar.activation(out=qhat[:, mc, :], in_=ps, func=AF.Identity,
                                     bias=bq_sb[:, mc:mc + 1], scale=1.0)

            # ---- phase 1b: v~ (keys on partitions) ----
            vhat = big.tile([P, KC, H * VW], bf, tag="vhat")
            nc.vector.memset(
                vhat.rearrange("p c (h w) -> p c h w", w=VW)[:, :, :, DK:DK + 1], 1.0)
            vt_sb = []
            vt_src = vt_d.ap().rearrange("(o p) s -> p o s", p=P)
            for half in range(2):
                t = xin.tile([P, KO, ROWS], bf, tag="xin")
                for ko in range(KO):
                    nc.sync.dma_start(
                        out=t[:, ko, :],
                        in_=vt_src[:, ko, half * ROWS:(half + 1) * ROWS])
                vt_sb.append(t)
            def vproj(kc):
                half, c = divmod(kc, KC // 2)
                ps = mm.tile([P, D], mybir.dt.float32, tag="mm")
                mm_acc(ps, D,
                       lambda ko, half=half, c=c: vt_sb[half][:, ko, c * P:(c + 1) * P],
                       lambda ko: w_sb["wv"][:, ko, :], KO)
                dst = vhat.rearrange("p c (h w) -> p c h w", w=VW)[:, kc, :, 0:DK]
                nc.vector.tensor_tensor(
                    out=dst, in0=ps.rearrange("p (h w) -> p h w", w=DK),
                    in1=bvb.rearrange("p (h w) -> p h w", w=DK), op=OP.add)

            # ---- phase 1c + 2: k~ per 2-head chunk, then attention ----
            for c in range(4, KC):
                nc.sync.dma_start(out=keep_sb[:, c, :], in_=keep_src[:, c, :])
            load_w("wo")

            ctxT = big.tile([P, MC, ROWS], bf, tag="ctxT")
            vh4 = vhat.rearrange("p c (h w) -> p c h w", w=VW)

            for mc in range(MC):
                kbuf = kp.tile([P, S], bf, tag="kbuf")
                for half in range(2):
                    ps = mm.tile([P, ROWS], mybir.dt.float32, tag="mm")
                    mm_acc(ps, ROWS,
                           lambda ko, mc=mc: w_sb["wk"][:, ko, mc * P:(mc + 1) * P],
                           lambda ko, half=half: kt_sb[half][:, ko, :], KO)
                    nc.scalar.activation(out=kbuf[:, half * ROWS:(half + 1) * ROWS],
                                         in_=ps, func=AF.Identity,
                                         bias=bk_sb[:, mc:mc + 1], scale=1.0)
                for hh in range(2):
                    h = 2 * mc + hh
                    pr = slice(hh * DK, (hh + 1) * DK)
                    ctx_ps = cx.tile([VW, ROWS], mybir.dt.float32, tag="cx")
                    for kc in range(KC):
                        if h == 0:
                            vproj(kc)
                        s_ps = mm.tile([P, ROWS], mybir.dt.float32, tag="mm")
                        for n0 in range(0, ROWS, NSPLIT):
                            nc.tensor.matmul(s_ps[:, n0:n0 + NSPLIT],
                                             kbuf[pr, kc * P:(kc + 1) * P],
                                             qhat[pr, mc, n0:n0 + NSPLIT],
                                             start=True, stop=True)
                        p_t = ppool.tile([P, ROWS], bf, tag="p")
                        nc.scalar.activation(out=p_t, in_=s_ps, func=AF.Exp,
                                             scale=1.0 / np.sqrt(DK))
                        nc.vector.tensor_tensor(out=p_t, in0=p_t,
                                                in1=keep_sb[:, kc, :], op=OP.mult)
                        for n0 in range(0, ROWS, NSPLIT):
                            nc.tensor.matmul(ctx_ps[:, n0:n0 + NSPLIT],
                                             vh4[:, kc, h, :], p_t[:, n0:n0 + NSPLIT],
                                             start=(kc == 0), stop=(kc == KC - 1))
                    # rowsum reciprocal: extract -> DRAM -> [128,8] transpose ->
                    # cheap reciprocal -> DRAM -> broadcast to 64 partitions.
                    rs_t = small.tile([1, ROWS], f32, tag="rs")
                    nc.vector.tensor_copy(out=rs_t, in_=ctx_ps[DK:DK + 1, :])
                    nc.sync.dma_start(out=rs_d[h].ap(), in_=rs_t)
                    rsT = small.tile([P, ROWS // P], f32, tag="rsT")
                    nc.sync.dma_start(
                        out=rsT, in_=rs_d[h].ap().rearrange("(p o) -> p o", p=P))
                    rsTb = small.tile([P, ROWS // P], bf, tag="rsTb")
                    with nc.allow_low_precision(reason="softmax rowsum recip in bf16"):
                        nc.vector.reciprocal(out=rsTb, in_=rsT)
                    nc.sync.dma_start(
                        out=rs2_d[h].ap().rearrange("(p o) -> p o", p=P), in_=rsTb)
                    rsb_t = small.tile([DK, ROWS], bf, tag="rsb")
                    nc.sync.dma_start(
                        out=rsb_t,
                        in_=bass.AP(tensor=rs2_d[h].ap().tensor, offset=0,
                                    ap=[[0, DK], [1, ROWS]]))
                    nc.vector.tensor_tensor(out=ctxT[pr, mc, :],
                                            in0=ctx_ps[0:DK, :], in1=rsb_t,
                                            op=OP.mult)

            # ---- phase 3: out projection + residual + LayerNorm ----
            nsub = 3
            sub = D // nsub  # 256 <= BN_STATS_FMAX
            for rc in range(RC):
                ps = mm.tile([P, D], mybir.dt.float32, tag="mm")
                mm_acc(ps, D,
                       lambda ko, rc=rc: ctxT[:, ko, rc * P:(rc + 1) * P],
                       lambda ko: w_sb["wo"][:, ko, :], KO)
                qres_t = ph3.tile([P, D], f32, tag="qres")
                nc.sync.dma_start(out=qres_t,
                                  in_=qres_d.ap()[rc * P:(rc + 1) * P, :])
                x_t = ph3.tile([P, D], f32, tag="x")
                nc.vector.tensor_tensor(out=x_t, in0=ps, in1=qres_t, op=OP.add)
                stats = small.tile([P, nsub, 6], f32, tag="stats")
                for sg in range(nsub):
                    nc.vector.bn_stats(out=stats[:, sg, :],
                                       in_=x_t[:, sg * sub:(sg + 1) * sub])
                mv = small.tile([P, 2], f32, tag="mv")
                nc.vector.bn_aggr(out=mv, in_=stats)
                std_t = small.tile([P, 1], f32, tag="std")
                nc.scalar.activation(out=std_t, in_=mv[:, 1:2], func=AF.Sqrt,
                                     bias=eps_t, scale=1.0)
                nc.vector.reciprocal(out=std_t, in_=std_t)
                nc.vector.tensor_scalar(out=x_t, in0=x_t, scalar1=mv[:, 0:1],
                                        scalar2=std_t, op0=OP.subtract,
                                        op1=OP.mult)
                nc.vector.tensor_tensor(out=x_t, in0=x_t, in1=gb, op=OP.mult)
                nc.vector.tensor_tensor(out=x_t, in0=x_t, in1=bb, op=OP.add)
                nc.sync.dma_start(out=out_d.ap()[rc * P:(rc + 1) * P, :], in_=x_t)

    nc.compile()
    return nc


def _get_nc():
    if "nc" not in _cached:
        _cached["nc"] = _build()
    return _cached["nc"]


def _make_in_maps(Q, Kt, V, attn_mask, Wq, bq, Wk, bk, Wv, bv, Wo, bo, gamma, beta):
    f32 = np.float32
    w = {"wq": np.ascontiguousarray(Wq, f32).astype(BF16),
         "wk": np.ascontiguousarray(Wk, f32).astype(BF16),
         "wv": np.ascontiguousarray(Wv, f32).astype(BF16),
         "wo": np.ascontiguousarray(Wo, f32).astype(BF16)}
    b = {"bq": np.ascontiguousarray(bq, f32), "bk": np.ascontiguousarray(bk, f32),
         "bv": np.ascontiguousarray(bv, f32),
         "gamma": np.ascontiguousarray(gamma, f32),
         "beta": np.ascontiguousarray(beta, f32)}
    bo_f = np.asarray(bo, f32)
    in_maps = []
    for c in range(NCORES):
        bidx, half = divmod(c, 2)
        rows = slice(half * ROWS, (half + 1) * ROWS)
        m = {
            "qt": np.ascontiguousarray(Q[bidx, rows].T).astype(BF16),
            "kt": np.ascontiguousarray(Kt[bidx].T).astype(BF16),
            "vt": np.ascontiguousarray(V[bidx].T).astype(BF16),
            "keep": np.ascontiguousarray(
                (~attn_mask[bidx, rows]).T.astype(BF16)),
            "qres": np.ascontiguousarray(Q[bidx, rows], f32) + bo_f,
        }
        m.update(w)
        m.update(b)
        in_maps.append(m)
    return in_maps


def kernel(Q, K, V, attn_mask, Wq, bq, Wk, bk, Wv, bv, Wo, bo, gamma, beta,
           _profile=None):
    from concourse.bass_utils import run_bass_kernel_spmd

    nc = _get_nc()
    in_maps = _make_in_maps(np.asarray(Q, np.float32), np.asarray(K, np.float32),
                            np.asarray(V, np.float32), np.asarray(attn_mask),
                            Wq, bq, Wk, bk, Wv, bv, Wo, bo, gamma, beta)
    kwargs = dict(_profile) if _profile else {}
    res = run_bass_kernel_spmd(nc, in_maps, list(range(NCORES)), **kwargs)
    if _profile is not None:
        _cached["last_results"] = res
    out = np.empty((B, S, D), np.float32)
    for c, m in enumerate(res.results):
        bidx, half = divmod(c, 2)
        out[bidx, half * ROWS:(half + 1) * ROWS] = m["out"]
    return out
